# revision 22
# baseline (speedup 1.0000x reference)
"""Two-layer GAT (nn_GAT_82334523064895) on 8 TRN2 NeuronCores via Bass.

Strategy (8-way contiguous node sharding, SPMD single NEFF):
  1. x is transposed + bf16-cast on host so phase A streams it at line rate
     (no DMA-transpose); loads alternate between the SP and ACT HWDGE
     queues to overlap with the PE GEMM: hT = W1aug.T @ xT accumulated in
     PSUM over 64 k-chunks, W1aug = [W1 | W1@a_s1 | W1@a_d1].
  2. h rows packed as 36B bf16 table rows [h0..h15 | s | d] into a DRAM
     slab; a small AllGather (295 KB) replicates the packed table, which
     is then expanded locally (via SBUF) into a 256B-stride gather table.
  3. Edge phase as uniform-width padded ELL (W = 64 >= 1 + max in-degree):
     per 2-tile group one dma_gather (int64-typed 256B rows) fetches all
     h[src] rows; pad slots hit a sentinel row (s = -3e4 so exp -> 0).
     Softmax without max-shift; weighted sums as a few fused whole-group
     ops: multiply split across DVE and GpSimd, bf16 pairwise-tree adds
     (DVE 2x packed mode) + final f32 reduce.
  4. Layer 2 identically with 8B bf16 packed rows [o2c0 | o2c1 | s2 | d2]
     (h2aug = o1 @ W2aug on DVE; b2 folded in pre-aggregation, valid
     since sum(alpha) = 1).
  5. Unnormalized outputs returned; global min/max rescale done on host.
Host does index prep, x transpose/cast, final rescale.
"""

import numpy as np
import ml_dtypes

N = 8192
F = 8192
H = 16
C = 2
NCORES = 8
NSH = N // NCORES          # nodes per core
P = 128
NT = NSH // P              # dst tiles per core
AUG1 = H + 2               # h .. s, d
KCH = F // P               # k chunks
ROW1 = AUG1                # bf16 elems per packed L1 table row (36B)
ROW2 = 4                   # bf16 elems per packed L2 table row (8B)
GROW = 128                 # bf16 elems per expanded gather row (256B)
NSHE = NSH + 1             # slab rows: NSH nodes + 1 sentinel row
TROWS = NCORES * NSHE
SENTROW = NSH              # sentinel position = core-0 slab row NSH
GT = 2                     # dst tiles per gather group
NEG = 0.2
SENT = -30000.0


def _install_tilefix():
    """Split the Tile kernel-tail drain's sem waits across multiple drains
    (this walrus build rejects >1 sync wait on a CTRL instruction)."""
    import bass_rust
    from bass_rust import ScopedClock
    import concourse.tile as tile

    def _split_drain_and_barrier(self, tick_clock, wait_clock):
        nc = self.nc
        drain_inst = nc.sync.drain()
        wait_clock.add_sem_waits(
            drain_inst.ins, ScopedClock({None: tick_clock.global_clock})
        )
        si = drain_inst.ins.sync_info
        waits = list(si.on_wait) if si is not None else []
        if len(waits) > 1:
            si.on_wait = waits[:1]
            for i in range(1, len(waits)):
                d2 = nc.sync.drain()
                si2 = d2.ins.sync_info
                if si2 is None:
                    d2.ins.sync_info = bass_rust.SyncInfo(on_wait=[], on_update=[])
                    si2 = d2.ins.sync_info
                si2.on_wait = waits[i : i + 1]
        nc.all_engine_barrier()
        popped = nc._tile_sem_poison_stack.pop()
        assert popped is self._sem_poison
        nc.clear_and_free_semaphores(list(self.sems.allocated().values()))
        nc.all_engine_barrier()

    tile.TileContext._drain_and_barrier = _split_drain_and_barrier


def _split_multiwaits(d):
    """Walrus in this build accepts a single sync wait per instruction; hoist
    extra waits onto wait-only EventSemaphore carriers inserted just before."""
    n = 0
    for fn in d["functions"]:
        for blk in fn["blocks"]:
            newl = []
            for ins in blk["instructions"]:
                si = ins.get("sync_info")
                waits = (si or {}).get("on_wait") or []
                if len(waits) > 1:
                    for w in waits[:-1]:
                        n += 1
                        newl.append(
                            {
                                "debug": ins.get("debug"),
                                "engine": ins["engine"],
                                "ins": [],
                                "outs": [],
                                "name": f"{ins['name']}-ws{n}",
                                "opcode": "EventSemaphore",
                                "sync_info": {"on_update": [], "on_wait": [w]},
                            }
                        )
                    si["on_wait"] = [waits[-1]]
                newl.append(ins)
            blk["instructions"] = newl
    return d


def _patch_serialization(nc):
    import types
    import json

    orig = nc.to_json_bytes

    def to_json_bytes_patched(self):
        d = json.loads(orig())
        _split_multiwaits(d)
        return json.dumps(d).encode()

    nc.to_json_bytes = types.MethodType(to_json_bytes_patched, nc)


def _build(W, phase="full"):
    import concourse.bass as bass
    import concourse.bacc as bacc
    import concourse.mybir as mybir
    import concourse.tile as tile
    from concourse.masks import make_identity

    _install_tilefix()
    dt = mybir.dt
    Alu = mybir.AluOpType
    Act = mybir.ActivationFunctionType
    RG = [list(range(NCORES))]
    NG = NT // GT              # gather groups
    WG = GT * W                # slots per group
    NIG = P * WG               # idxs per group
    SROWS = TROWS // P + 1     # staging rows per partition (ceil 8200/128)

    nc = bacc.Bacc("TRN2", debug=False, dynamic_dma_scratch_size=32768)
    xs_p = nc.declare_dram_parameter("xs", [F, NSH], dt.bfloat16, isOutput=False)
    w1_p = nc.declare_dram_parameter("w1s", [P, KCH * AUG1], dt.bfloat16, isOutput=False)
    v1_p = nc.declare_dram_parameter("v1", [AUG1, 1], dt.float32, isOutput=False)
    idx1_p = nc.declare_dram_parameter("idx1", [P, NG * NIG // 16], dt.int16, isOutput=False)
    w2b_p = nc.declare_dram_parameter("w2b", [P, 4 * H], dt.float32, isOutput=False)
    b2a_p = nc.declare_dram_parameter("b2a", [P, 4], dt.float32, isOutput=False)
    out_p = nc.declare_dram_parameter("out", [NSH, C], dt.float32, isOutput=True)

    with tile.TileContext(nc) as tc:
        with (
            tc.tile_pool(name="const", bufs=1) as cpool,
            tc.tile_pool(name="xload", bufs=4) as xpool,
            tc.tile_pool(name="work", bufs=1) as wpool,
            tc.tile_pool(name="gath", bufs=2) as gpool,
            tc.tile_pool(name="pst", bufs=2, space="PSUM") as ppool,
            tc.tile_pool(name="psacc", bufs=1, space="PSUM") as psacc,
            tc.tile_pool(name="dram", bufs=1, space="DRAM") as dpool,
        ):
            def _emit():
                # ---- constants needed before/during phase A (SP queue head)
                w1_s = cpool.tile([P, KCH * AUG1], dt.bfloat16)
                nc.sync.dma_start(w1_s[:], w1_p[:])
                w1v = w1_s[:].rearrange("p (c f) -> p c f", f=AUG1)
                v1_s = cpool.tile([AUG1, 1], dt.float32)
                nc.sync.dma_start(v1_s[:], v1_p[:])
                ident = cpool.tile([P, P], dt.float32)
                make_identity(nc, ident[:])
                sent1 = cpool.tile([1, ROW1], dt.bfloat16)
                nc.gpsimd.memset(sent1[:], SENT)
                sent2 = cpool.tile([1, ROW2], dt.bfloat16)
                nc.gpsimd.memset(sent2[:], SENT)

                # ---- internal DRAM
                l1slab = dpool.tile([NSHE, ROW1], dt.bfloat16)
                table1 = dpool.tile([TROWS, ROW1], dt.bfloat16, addr_space="Shared")
                gtab1 = dpool.tile([TROWS, GROW], dt.bfloat16)
                l2slab = dpool.tile([NSHE, ROW2], dt.bfloat16)
                table2 = dpool.tile([TROWS, ROW2], dt.bfloat16, addr_space="Shared")
                gtab2 = dpool.tile([TROWS, GROW], dt.bfloat16)

                # ---- Phase A: hT = W1aug.T @ xT accumulated over k chunks;
                # x loads alternate between the SP and ACT HWDGE queues.
                qs = [nc.sync, nc.scalar]
                hps0 = psacc.tile([AUG1, 512], dt.float32, tag="hps0")
                hps1 = psacc.tile([AUG1, 512], dt.float32, tag="hps1")
                hps = [hps0, hps1]
                for ck in range(KCH // 2):
                    xt = xpool.tile([P, 2, NSH], dt.bfloat16, tag="xt")
                    qs[ck % 2].dma_start(
                        xt[:],
                        xs_p[ck * 2 * P : (ck + 1) * 2 * P, :].rearrange(
                            "(j p) n -> p j n", p=P
                        ),
                    )
                    for j in range(2):
                        c2 = 2 * ck + j
                        for mh in range(2):
                            nc.tensor.matmul(
                                hps[mh][:],
                                w1v[:, c2, :],
                                xt[:, j, mh * 512 : (mh + 1) * 512],
                                start=(c2 == 0),
                                stop=(c2 == KCH - 1),
                            )

                # constants only needed from the edge phase on
                idx1_s = cpool.tile([P, NG * NIG // 16], dt.int16)
                nc.sync.dma_start(idx1_s[:], idx1_p[:])
                idx2_s = idx1_s
                w2b_s = cpool.tile([P, 4, H], dt.float32)
                nc.sync.dma_start(w2b_s[:], w2b_p[:].rearrange("p (c k) -> p c k", k=H))
                b2a_s = cpool.tile([P, 4], dt.float32)
                nc.sync.dma_start(b2a_s[:], b2a_p[:])

                hT = cpool.tile([AUG1, NSH], dt.float32)
                for mh in range(2):
                    nc.scalar.activation(
                        hT[:, mh * 512 : (mh + 1) * 512],
                        hps[mh][:],
                        Act.Identity,
                        bias=v1_s[:],
                    )

                # ---- Phase B: pack h rows (all bf16) -> slab -> AllGather ->
                # local expand into the 256B-stride gather table.
                slabsb = cpool.tile([P, NT, ROW1], dt.bfloat16)
                for t in range(NT):
                    hr_ps = ppool.tile([P, AUG1], dt.float32, tag="hrps")
                    nc.tensor.transpose(
                        hr_ps[:], hT[:, t * P : (t + 1) * P], ident[:AUG1, :AUG1]
                    )
                    nc.vector.tensor_copy(slabsb[:, t, :], hr_ps[:])
                nc.sync.dma_start(
                    l1slab[0:NSH, :].rearrange("(t p) f -> p t f", p=P), slabsb[:]
                )
                nc.sync.dma_start(l1slab[NSH : NSH + 1, :], sent1[:])

                def dump2(view):
                    st = wpool.tile([P, NT, C], dt.float32, tag="dump")
                    nc.vector.tensor_copy(st[:], view)
                    nc.sync.dma_start(
                        out_p[:].rearrange("(t p) c -> p t c", p=P), st[:]
                    )

                def expand(table, gtab, ROWB):
                    # packed [TROWS, ROWB] -> 256B-stride [TROWS, GROW] cols 0:ROWB
                    stage = wpool.tile([P, SROWS, ROWB], dt.bfloat16, tag=f"stage{ROWB}")
                    nc.sync.dma_start(
                        stage[:, 0 : SROWS - 1, :],
                        table[0 : P * (SROWS - 1), :].rearrange(
                            "(p j) f -> p j f", p=P
                        ),
                    )
                    rem = TROWS - P * (SROWS - 1)
                    nc.sync.dma_start(
                        stage[0:rem, SROWS - 1, :], table[P * (SROWS - 1) : TROWS, :]
                    )
                    nc.sync.dma_start(
                        gtab[0 : P * (SROWS - 1), 0:ROWB].rearrange(
                            "(p j) f -> p j f", p=P
                        ),
                        stage[:, 0 : SROWS - 1, :],
                    )
                    nc.sync.dma_start(
                        gtab[P * (SROWS - 1) : TROWS, 0:ROWB],
                        stage[0:rem, SROWS - 1, :],
                    )

                if phase == "gemm":
                    dump2(slabsb[:, :, 0:C])
                    return
                nc.gpsimd.collective_compute(
                    "AllGather",
                    Alu.bypass,
                    replica_groups=RG,
                    ins=[l1slab[:].opt()],
                    outs=[table1[0:TROWS, :].opt()],
                )
                if phase == "ag1":
                    tt = wpool.tile([P, NT, ROW1], dt.bfloat16, tag="dumpt")
                    nc.sync.dma_start(
                        tt[:], table1[0:NSH, :].rearrange("(t p) f -> p t f", p=P)
                    )
                    dump2(tt[:, :, 0:C])
                    return
                expand(table1, gtab1, ROW1)

                def edge_group(gtab, idx_s, g, layer):
                    """Gather + softmax-weighted aggregation for dst tiles
                    [g*GT, (g+1)*GT). Returns o (aggregated) [P, GT, FH]."""
                    FH = H if layer == 1 else C
                    G = gpool.tile([P, WG, 32], dt.int64, tag="G")
                    nc.gpsimd.dma_gather(
                        out_ap=G[:],
                        in_ap=gtab[:].bitcast(dt.int64),
                        idxs_ap=idx_s[:, g * (NIG // 16) : (g + 1) * (NIG // 16)],
                        num_idxs=NIG,
                        num_idxs_reg=NIG,
                        elem_size=32,
                        single_packet=False,
                    )
                    Gb = (
                        G[:]
                        .bitcast(dt.bfloat16)
                        .rearrange("p (t w) f -> p t w f", w=W)
                    )  # [P, GT, W, GROW]
                    scol, dcol = FH, FH + 1
                    z = wpool.tile([P, GT, W], dt.float32, tag=f"z{layer}")
                    nc.vector.tensor_tensor(
                        out=z[:],
                        in0=Gb[:, :, :, scol : scol + 1].squeeze(),
                        in1=Gb[:, :, 0:1, dcol : dcol + 1]
                        .rearrange("p t a b -> p t (a b)")
                        .to_broadcast([P, GT, W]),
                        op=Alu.add,
                    )
                    e = wpool.tile([P, GT, W], dt.float32, tag=f"e{layer}")
                    nc.vector.scalar_tensor_tensor(
                        out=e[:], in0=z[:], scalar=NEG, in1=z[:],
                        op0=Alu.mult, op1=Alu.max,
                    )
                    exb = wpool.tile([P, GT, W], dt.bfloat16, tag=f"exb{layer}")
                    nc.scalar.activation(exb[:], e[:], Act.Exp)
                    den = wpool.tile([P, GT], dt.float32, tag=f"den{layer}")
                    nc.vector.tensor_reduce(
                        den[:], exb[:], axis=mybir.AxisListType.X, op=Alu.add
                    )
                    rec = wpool.tile([P, GT], dt.float32, tag=f"rec{layer}")
                    nc.vector.reciprocal(rec[:], den[:])
                    # num[p,t,f] = sum_w exb * h ; multiply split DVE/Pool,
                    # bf16 tree-add (DVE 2x) + final f32 reduce
                    hv = Gb[:, :, :, 0:FH].rearrange("p t w f -> p t f w")
                    tmp = wpool.tile([P, GT, FH, W], dt.bfloat16, tag=f"tm{layer}")
                    FB = FH // 2
                    for eng, f0, fn in (
                        (nc.vector, 0, FB),
                        (nc.gpsimd, FB, FH),
                    ):
                        eng.tensor_tensor(
                            out=tmp[:, :, f0:fn, :],
                            in0=hv[:, :, f0:fn, :],
                            in1=exb[:].unsqueeze(2).to_broadcast(
                                [P, GT, fn - f0, W]
                            ),
                            op=Alu.mult,
                        )
                    src = tmp[:]
                    w = W
                    while w > 8:
                        w //= 2
                        half = wpool.tile(
                            [P, GT, FH, w], dt.bfloat16, tag=f"tr{layer}_{w}"
                        )
                        nc.vector.tensor_tensor(
                            out=half[:],
                            in0=src[:, :, :, 0:w],
                            in1=src[:, :, :, w : 2 * w],
                            op=Alu.add,
                        )
                        src = half[:]
                    num = wpool.tile([P, GT, FH], dt.float32, tag=f"num{layer}")
                    nc.vector.tensor_reduce(
                        num[:], src, axis=mybir.AxisListType.X, op=Alu.add
                    )
                    o = wpool.tile([P, GT, FH], dt.float32, tag=f"o{layer}")
                    nc.vector.tensor_tensor(
                        out=o[:],
                        in0=num[:],
                        in1=rec[:].unsqueeze(2).to_broadcast([P, GT, FH]),
                        op=Alu.mult,
                    )
                    return o

                # ---- Phase C: layer-1 edge aggregation + h2 slab
                h2all = cpool.tile([P, NT, 4], dt.bfloat16)
                oall = cpool.tile([P, NT, C], dt.float32)
                for g in range(NG):
                    o1 = edge_group(gtab1, idx1_s, g, 1)
                    if phase == "gonly":
                        nc.vector.tensor_copy(
                            oall[:, g * GT : (g + 1) * GT, :], o1[:, :, 0:C]
                        )
                        continue
                    # h2aug = o1 @ W2aug + [b2|0]  (per-partition matvec)
                    tmp2 = wpool.tile([P, GT, 4, H], dt.float32, tag="tmp2")
                    nc.vector.tensor_tensor(
                        out=tmp2[:],
                        in0=o1[:].unsqueeze(2).to_broadcast([P, GT, 4, H]),
                        in1=w2b_s[:].unsqueeze(1).to_broadcast([P, GT, 4, H]),
                        op=Alu.mult,
                    )
                    h2t = wpool.tile([P, GT, 4], dt.float32, tag="h2t")
                    nc.vector.tensor_reduce(
                        h2t[:], tmp2[:], axis=mybir.AxisListType.X, op=Alu.add
                    )
                    nc.vector.tensor_tensor(
                        out=h2all[:, g * GT : (g + 1) * GT, :],
                        in0=h2t[:],
                        in1=b2a_s[:].unsqueeze(1).to_broadcast([P, GT, 4]),
                        op=Alu.add,
                    )
                if phase == "gonly":
                    dump2(oall[:])
                    return
                nc.sync.dma_start(
                    l2slab[0:NSH, :].rearrange("(t p) f -> p t f", p=P), h2all[:]
                )
                nc.sync.dma_start(l2slab[NSH : NSH + 1, :], sent2[:])
                if phase == "gat1":
                    dump2(h2all[:, :, 0:C])
                    return
                nc.gpsimd.collective_compute(
                    "AllGather",
                    Alu.bypass,
                    replica_groups=RG,
                    ins=[l2slab[:].opt()],
                    outs=[table2[0:TROWS, :].opt()],
                )
                if phase == "ag2":
                    tt = wpool.tile([P, NT, ROW2], dt.bfloat16, tag="dumpt2")
                    nc.sync.dma_start(
                        tt[:], table2[0:NSH, :].rearrange("(t p) f -> p t f", p=P)
                    )
                    dump2(tt[:, :, 0:C])
                    return
                expand(table2, gtab2, ROW2)

                # ---- Phase D: layer-2 edge aggregation -> unnormalized out
                allout = cpool.tile([P, NT, C], dt.float32)
                for g in range(NG):
                    o2 = edge_group(gtab2, idx2_s, g, 2)
                    nc.vector.tensor_copy(allout[:, g * GT : (g + 1) * GT, :], o2[:])
                nc.sync.dma_start(
                    out_p[:].rearrange("(t p) c -> p t c", p=P), allout[:]
                )

            _emit()
    nc.compile()
    _patch_serialization(nc)
    return nc


def _prep(x, edge_index, W1, a_src1, a_dst1, b1, W2, a_src2, a_dst2, b2):
    ei = np.asarray(edge_index).astype(np.int64)
    src_all, dst_all = ei[0], ei[1]
    E = src_all.shape[0]
    counts = np.bincount(dst_all, minlength=N)
    W = 64  # pow2 ELL width (pairwise-tree friendly); widen if degree demands
    while W < int(counts.max()) + 1:
        W *= 2

    # table row position of node g: NSHE*(g//NSH) + g%NSH; sentinel at SENTROW
    g = np.arange(N, dtype=np.int64)
    pos = NSHE * (g // NSH) + (g % NSH)

    perm_e = np.argsort(dst_all, kind="stable")
    dsorted = dst_all[perm_e]
    ssorted = src_all[perm_e]
    starts = np.zeros(N + 1, np.int64)
    np.cumsum(counts, out=starts[1:])
    rank = np.arange(E, dtype=np.int64) - starts[dsorted]

    mat = np.full((N, W), SENTROW, np.int64)
    mat[:, 0] = pos
    mat[dsorted, 1 + rank] = pos[ssorted]

    # dma_gather idx order per group: i = (t_in_group*W + w)*128 + p
    idx_maps = []
    for c in range(NCORES):
        m = mat[c * NSH : (c + 1) * NSH].reshape(NT // GT, GT, P, W)
        m = m.transpose(0, 1, 3, 2).reshape(-1)  # [g, t, w, p] flattened
        idx_maps.append(np.tile(m.reshape(-1, 16).T, (8, 1)).astype(np.int16))

    bf = ml_dtypes.bfloat16
    W1aug = np.concatenate(
        [W1, (W1 @ a_src1)[:, None], (W1 @ a_dst1)[:, None]], axis=1
    ).astype(np.float32)
    w1s = (
        W1aug.reshape(KCH, P, AUG1).transpose(1, 0, 2).reshape(P, KCH * AUG1)
    ).astype(bf)
    v1 = np.concatenate([b1.astype(np.float32), np.zeros(2, np.float32)]).reshape(
        AUG1, 1
    )
    W2aug = np.concatenate(
        [W2, (W2 @ a_src2)[:, None], (W2 @ a_dst2)[:, None]], axis=1
    ).astype(np.float32)
    w2b = np.tile(W2aug.T.reshape(1, 4 * H), (P, 1)).astype(np.float32)
    b2a = np.tile(
        np.array([b2[0], b2[1], 0.0, 0.0], np.float32), (P, 1)
    ).astype(np.float32)

    xb = np.asarray(x, np.float32).astype(bf)
    in_maps = []
    for c in range(NCORES):
        in_maps.append(
            {
                "xs": np.ascontiguousarray(xb[NSH * c : NSH * (c + 1)].T),
                "w1s": w1s,
                "v1": v1,
                "idx1": idx_maps[c],
                "w2b": w2b,
                "b2a": b2a,
            }
        )
    return W, in_maps


_NC_CACHE = {}


def _get_nc(W):
    if W not in _NC_CACHE:
        _NC_CACHE[W] = _build(W)
    return _NC_CACHE[W]


def kernel(**inputs):
    from concourse.bass_utils import run_bass_kernel_spmd

    W, in_maps = _prep(
        inputs["x"], inputs["edge_index"], inputs["W1"], inputs["a_src1"],
        inputs["a_dst1"], inputs["b1"], inputs["W2"], inputs["a_src2"],
        inputs["a_dst2"], inputs["b2"],
    )
    nc = _get_nc(W)
    res = run_bass_kernel_spmd(nc, in_maps, list(range(NCORES)))
    out = np.concatenate([res.results[c]["out"] for c in range(NCORES)], axis=0)
    out = out.astype(np.float64)
    mn, mx = out.min(), out.max()
    return (2.0 * (out - mn) / (mx - mn) - 1.0).astype(np.float32)


# revision 23
# speedup vs baseline: 1.0125x; 1.0125x over previous
"""Two-layer GAT (nn_GAT_82334523064895) on 8 TRN2 NeuronCores via Bass.

Strategy (8-way contiguous node sharding, SPMD single NEFF):
  1. x is transposed + bf16-cast on host so phase A streams it at line rate
     (no DMA-transpose); loads alternate between the SP and ACT HWDGE
     queues to overlap with the PE GEMM: hT = W1aug.T @ xT accumulated in
     PSUM over 64 k-chunks, W1aug = [W1 | W1@a_s1 | W1@a_d1].
  2. h rows packed as 36B bf16 table rows [h0..h15 | s | d] into a DRAM
     slab; a small AllGather (295 KB) replicates the packed table, which
     is then expanded locally (via SBUF) into a 256B-stride gather table.
  3. Edge phase as uniform-width padded ELL (W = 64 >= 1 + max in-degree):
     per 2-tile group one dma_gather (int64-typed 256B rows) fetches all
     h[src] rows; pad slots hit a sentinel row (s = -3e4 so exp -> 0).
     Softmax without max-shift; weighted sums as a few fused whole-group
     ops: multiply split across DVE and GpSimd, bf16 pairwise-tree adds
     (DVE 2x packed mode) + final f32 reduce.
  4. Layer 2 identically with 8B bf16 packed rows [o2c0 | o2c1 | s2 | d2]
     (h2aug = o1 @ W2aug on DVE; b2 folded in pre-aggregation, valid
     since sum(alpha) = 1).
  5. Unnormalized outputs returned; global min/max rescale done on host.
Host does index prep, x transpose/cast, final rescale.
"""

import numpy as np
import ml_dtypes

N = 8192
F = 8192
H = 16
C = 2
NCORES = 8
NSH = N // NCORES          # nodes per core
P = 128
NT = NSH // P              # dst tiles per core
AUG1 = H + 2               # h .. s, d
KCH = F // P               # k chunks
ROW1 = AUG1                # bf16 elems per packed L1 table row (36B)
ROW2 = 4                   # bf16 elems per packed L2 table row (8B)
GROW = 128                 # bf16 elems per expanded gather row (256B)
NSHE = NSH + 1             # slab rows: NSH nodes + 1 sentinel row
TROWS = NCORES * NSHE
SENTROW = NSH              # sentinel position = core-0 slab row NSH
GT = 1                     # dst tiles per gather group
NEG = 0.2
SENT = -30000.0


def _install_tilefix():
    """Split the Tile kernel-tail drain's sem waits across multiple drains
    (this walrus build rejects >1 sync wait on a CTRL instruction)."""
    import bass_rust
    from bass_rust import ScopedClock
    import concourse.tile as tile

    def _split_drain_and_barrier(self, tick_clock, wait_clock):
        nc = self.nc
        drain_inst = nc.sync.drain()
        wait_clock.add_sem_waits(
            drain_inst.ins, ScopedClock({None: tick_clock.global_clock})
        )
        si = drain_inst.ins.sync_info
        waits = list(si.on_wait) if si is not None else []
        if len(waits) > 1:
            si.on_wait = waits[:1]
            for i in range(1, len(waits)):
                d2 = nc.sync.drain()
                si2 = d2.ins.sync_info
                if si2 is None:
                    d2.ins.sync_info = bass_rust.SyncInfo(on_wait=[], on_update=[])
                    si2 = d2.ins.sync_info
                si2.on_wait = waits[i : i + 1]
        nc.all_engine_barrier()
        popped = nc._tile_sem_poison_stack.pop()
        assert popped is self._sem_poison
        nc.clear_and_free_semaphores(list(self.sems.allocated().values()))
        nc.all_engine_barrier()

    tile.TileContext._drain_and_barrier = _split_drain_and_barrier


def _split_multiwaits(d):
    """Walrus in this build accepts a single sync wait per instruction; hoist
    extra waits onto wait-only EventSemaphore carriers inserted just before."""
    n = 0
    for fn in d["functions"]:
        for blk in fn["blocks"]:
            newl = []
            for ins in blk["instructions"]:
                si = ins.get("sync_info")
                waits = (si or {}).get("on_wait") or []
                if len(waits) > 1:
                    for w in waits[:-1]:
                        n += 1
                        newl.append(
                            {
                                "debug": ins.get("debug"),
                                "engine": ins["engine"],
                                "ins": [],
                                "outs": [],
                                "name": f"{ins['name']}-ws{n}",
                                "opcode": "EventSemaphore",
                                "sync_info": {"on_update": [], "on_wait": [w]},
                            }
                        )
                    si["on_wait"] = [waits[-1]]
                newl.append(ins)
            blk["instructions"] = newl
    return d


def _patch_serialization(nc):
    import types
    import json

    orig = nc.to_json_bytes

    def to_json_bytes_patched(self):
        d = json.loads(orig())
        _split_multiwaits(d)
        return json.dumps(d).encode()

    nc.to_json_bytes = types.MethodType(to_json_bytes_patched, nc)


def _build(W, phase="full"):
    import concourse.bass as bass
    import concourse.bacc as bacc
    import concourse.mybir as mybir
    import concourse.tile as tile
    from concourse.masks import make_identity

    _install_tilefix()
    dt = mybir.dt
    Alu = mybir.AluOpType
    Act = mybir.ActivationFunctionType
    RG = [list(range(NCORES))]
    NG = NT // GT              # gather groups
    WG = GT * W                # slots per group
    NIG = P * WG               # idxs per group
    SROWS = TROWS // P + 1     # staging rows per partition (ceil 8200/128)

    nc = bacc.Bacc("TRN2", debug=False)
    xs_p = nc.declare_dram_parameter("xs", [F, NSH], dt.bfloat16, isOutput=False)
    w1_p = nc.declare_dram_parameter("w1s", [P, KCH * AUG1], dt.bfloat16, isOutput=False)
    v1_p = nc.declare_dram_parameter("v1", [AUG1, 1], dt.float32, isOutput=False)
    idx1_p = nc.declare_dram_parameter("idx1", [P, NG * NIG // 16], dt.int16, isOutput=False)
    w2b_p = nc.declare_dram_parameter("w2b", [P, 4 * H], dt.float32, isOutput=False)
    b2a_p = nc.declare_dram_parameter("b2a", [P, 4], dt.float32, isOutput=False)
    out_p = nc.declare_dram_parameter("out", [NSH, C], dt.float32, isOutput=True)

    with tile.TileContext(nc) as tc:
        with (
            tc.tile_pool(name="const", bufs=1) as cpool,
            tc.tile_pool(name="xload", bufs=4) as xpool,
            tc.tile_pool(name="work", bufs=1) as wpool,
            tc.tile_pool(name="gath", bufs=2) as gpool,
            tc.tile_pool(name="pst", bufs=2, space="PSUM") as ppool,
            tc.tile_pool(name="psacc", bufs=1, space="PSUM") as psacc,
            tc.tile_pool(name="dram", bufs=1, space="DRAM") as dpool,
        ):
            def _emit():
                # ---- constants needed before/during phase A (SP queue head)
                w1_s = cpool.tile([P, KCH * AUG1], dt.bfloat16)
                nc.sync.dma_start(w1_s[:], w1_p[:])
                w1v = w1_s[:].rearrange("p (c f) -> p c f", f=AUG1)
                v1_s = cpool.tile([AUG1, 1], dt.float32)
                nc.sync.dma_start(v1_s[:], v1_p[:])
                ident = cpool.tile([P, P], dt.float32)
                make_identity(nc, ident[:])
                sent1 = cpool.tile([1, ROW1], dt.bfloat16)
                nc.gpsimd.memset(sent1[:], SENT)
                sent2 = cpool.tile([1, ROW2], dt.bfloat16)
                nc.gpsimd.memset(sent2[:], SENT)

                # ---- internal DRAM
                l1slab = dpool.tile([NSHE, ROW1], dt.bfloat16)
                table1 = dpool.tile([TROWS, ROW1], dt.bfloat16, addr_space="Shared")
                gtab1 = dpool.tile([TROWS, GROW], dt.bfloat16)
                l2slab = dpool.tile([NSHE, ROW2], dt.bfloat16)
                table2 = dpool.tile([TROWS, ROW2], dt.bfloat16, addr_space="Shared")
                gtab2 = dpool.tile([TROWS, GROW], dt.bfloat16)

                # ---- Phase A: hT = W1aug.T @ xT accumulated over k chunks;
                # x loads alternate between the SP and ACT HWDGE queues.
                qs = [nc.sync, nc.scalar]
                hps0 = psacc.tile([AUG1, 512], dt.float32, tag="hps0")
                hps1 = psacc.tile([AUG1, 512], dt.float32, tag="hps1")
                hps = [hps0, hps1]
                for ck in range(KCH // 2):
                    xt = xpool.tile([P, 2, NSH], dt.bfloat16, tag="xt")
                    qs[ck % 2].dma_start(
                        xt[:],
                        xs_p[ck * 2 * P : (ck + 1) * 2 * P, :].rearrange(
                            "(j p) n -> p j n", p=P
                        ),
                    )
                    for j in range(2):
                        c2 = 2 * ck + j
                        for mh in range(2):
                            nc.tensor.matmul(
                                hps[mh][:],
                                w1v[:, c2, :],
                                xt[:, j, mh * 512 : (mh + 1) * 512],
                                start=(c2 == 0),
                                stop=(c2 == KCH - 1),
                            )

                # constants only needed from the edge phase on
                idx1_s = cpool.tile([P, NG * NIG // 16], dt.int16)
                nc.sync.dma_start(idx1_s[:], idx1_p[:])
                idx2_s = idx1_s
                w2b_s = cpool.tile([P, 4, H], dt.float32)
                nc.sync.dma_start(w2b_s[:], w2b_p[:].rearrange("p (c k) -> p c k", k=H))
                b2a_s = cpool.tile([P, 4], dt.float32)
                nc.sync.dma_start(b2a_s[:], b2a_p[:])

                hT = cpool.tile([AUG1, NSH], dt.float32)
                for mh in range(2):
                    nc.scalar.activation(
                        hT[:, mh * 512 : (mh + 1) * 512],
                        hps[mh][:],
                        Act.Identity,
                        bias=v1_s[:],
                    )

                # ---- Phase B: pack h rows (all bf16) -> slab -> AllGather ->
                # local expand into the 256B-stride gather table.
                slabsb = cpool.tile([P, NT, ROW1], dt.bfloat16)
                for t in range(NT):
                    hr_ps = ppool.tile([P, AUG1], dt.float32, tag="hrps")
                    nc.tensor.transpose(
                        hr_ps[:], hT[:, t * P : (t + 1) * P], ident[:AUG1, :AUG1]
                    )
                    nc.vector.tensor_copy(slabsb[:, t, :], hr_ps[:])
                nc.sync.dma_start(
                    l1slab[0:NSH, :].rearrange("(t p) f -> p t f", p=P), slabsb[:]
                )
                nc.sync.dma_start(l1slab[NSH : NSH + 1, :], sent1[:])

                def dump2(view):
                    st = wpool.tile([P, NT, C], dt.float32, tag="dump")
                    nc.vector.tensor_copy(st[:], view)
                    nc.sync.dma_start(
                        out_p[:].rearrange("(t p) c -> p t c", p=P), st[:]
                    )

                def expand(table, gtab, ROWB):
                    # packed [TROWS, ROWB] -> 256B-stride [TROWS, GROW] cols 0:ROWB
                    stage = wpool.tile([P, SROWS, ROWB], dt.bfloat16, tag=f"stage{ROWB}")
                    nc.sync.dma_start(
                        stage[:, 0 : SROWS - 1, :],
                        table[0 : P * (SROWS - 1), :].rearrange(
                            "(p j) f -> p j f", p=P
                        ),
                    )
                    rem = TROWS - P * (SROWS - 1)
                    nc.sync.dma_start(
                        stage[0:rem, SROWS - 1, :], table[P * (SROWS - 1) : TROWS, :]
                    )
                    nc.sync.dma_start(
                        gtab[0 : P * (SROWS - 1), 0:ROWB].rearrange(
                            "(p j) f -> p j f", p=P
                        ),
                        stage[:, 0 : SROWS - 1, :],
                    )
                    nc.sync.dma_start(
                        gtab[P * (SROWS - 1) : TROWS, 0:ROWB],
                        stage[0:rem, SROWS - 1, :],
                    )

                if phase == "gemm":
                    dump2(slabsb[:, :, 0:C])
                    return
                nc.gpsimd.collective_compute(
                    "AllGather",
                    Alu.bypass,
                    replica_groups=RG,
                    ins=[l1slab[:].opt()],
                    outs=[table1[0:TROWS, :].opt()],
                )
                if phase == "ag1":
                    tt = wpool.tile([P, NT, ROW1], dt.bfloat16, tag="dumpt")
                    nc.sync.dma_start(
                        tt[:], table1[0:NSH, :].rearrange("(t p) f -> p t f", p=P)
                    )
                    dump2(tt[:, :, 0:C])
                    return
                expand(table1, gtab1, ROW1)

                def edge_group(gtab, idx_s, g, layer):
                    """Gather + softmax-weighted aggregation for dst tiles
                    [g*GT, (g+1)*GT). Returns o (aggregated) [P, GT, FH]."""
                    FH = H if layer == 1 else C
                    G = gpool.tile([P, WG, 32], dt.int64, tag="G")
                    nc.gpsimd.dma_gather(
                        out_ap=G[:],
                        in_ap=gtab[:].bitcast(dt.int64),
                        idxs_ap=idx_s[:, g * (NIG // 16) : (g + 1) * (NIG // 16)],
                        num_idxs=NIG,
                        num_idxs_reg=NIG,
                        elem_size=32,
                        single_packet=False,
                    )
                    Gb = (
                        G[:]
                        .bitcast(dt.bfloat16)
                        .rearrange("p (t w) f -> p t w f", w=W)
                    )  # [P, GT, W, GROW]
                    scol, dcol = FH, FH + 1
                    z = wpool.tile([P, GT, W], dt.float32, tag=f"z{layer}")
                    nc.vector.tensor_tensor(
                        out=z[:],
                        in0=Gb[:, :, :, scol : scol + 1].squeeze(),
                        in1=Gb[:, :, 0:1, dcol : dcol + 1]
                        .rearrange("p t a b -> p t (a b)")
                        .to_broadcast([P, GT, W]),
                        op=Alu.add,
                    )
                    e = wpool.tile([P, GT, W], dt.float32, tag=f"e{layer}")
                    nc.vector.scalar_tensor_tensor(
                        out=e[:], in0=z[:], scalar=NEG, in1=z[:],
                        op0=Alu.mult, op1=Alu.max,
                    )
                    exb = wpool.tile([P, GT, W], dt.bfloat16, tag=f"exb{layer}")
                    nc.scalar.activation(exb[:], e[:], Act.Exp)
                    den = wpool.tile([P, GT], dt.float32, tag=f"den{layer}")
                    nc.vector.tensor_reduce(
                        den[:], exb[:], axis=mybir.AxisListType.X, op=Alu.add
                    )
                    rec = wpool.tile([P, GT], dt.float32, tag=f"rec{layer}")
                    nc.vector.reciprocal(rec[:], den[:])
                    # num[p,t,f] = sum_w exb * h ; multiply split DVE/Pool,
                    # bf16 tree-add (DVE 2x) + final f32 reduce
                    hv = Gb[:, :, :, 0:FH].rearrange("p t w f -> p t f w")
                    tmp = wpool.tile([P, GT, FH, W], dt.bfloat16, tag=f"tm{layer}")
                    FB = FH // 2
                    for eng, f0, fn in (
                        (nc.vector, 0, FB),
                        (nc.gpsimd, FB, FH),
                    ):
                        eng.tensor_tensor(
                            out=tmp[:, :, f0:fn, :],
                            in0=hv[:, :, f0:fn, :],
                            in1=exb[:].unsqueeze(2).to_broadcast(
                                [P, GT, fn - f0, W]
                            ),
                            op=Alu.mult,
                        )
                    src = tmp[:]
                    w = W
                    while w > 8:
                        w //= 2
                        half = wpool.tile(
                            [P, GT, FH, w], dt.bfloat16, tag=f"tr{layer}_{w}"
                        )
                        nc.vector.tensor_tensor(
                            out=half[:],
                            in0=src[:, :, :, 0:w],
                            in1=src[:, :, :, w : 2 * w],
                            op=Alu.add,
                        )
                        src = half[:]
                    num = wpool.tile([P, GT, FH], dt.float32, tag=f"num{layer}")
                    nc.vector.tensor_reduce(
                        num[:], src, axis=mybir.AxisListType.X, op=Alu.add
                    )
                    o = wpool.tile([P, GT, FH], dt.float32, tag=f"o{layer}")
                    nc.vector.tensor_tensor(
                        out=o[:],
                        in0=num[:],
                        in1=rec[:].unsqueeze(2).to_broadcast([P, GT, FH]),
                        op=Alu.mult,
                    )
                    return o

                # ---- Phase C: layer-1 edge aggregation + h2 slab
                h2all = cpool.tile([P, NT, 4], dt.bfloat16)
                oall = cpool.tile([P, NT, C], dt.float32)
                for g in range(NG):
                    o1 = edge_group(gtab1, idx1_s, g, 1)
                    if phase == "gonly":
                        nc.vector.tensor_copy(
                            oall[:, g * GT : (g + 1) * GT, :], o1[:, :, 0:C]
                        )
                        continue
                    # h2aug = o1 @ W2aug + [b2|0]  (per-partition matvec)
                    tmp2 = wpool.tile([P, GT, 4, H], dt.float32, tag="tmp2")
                    nc.vector.tensor_tensor(
                        out=tmp2[:],
                        in0=o1[:].unsqueeze(2).to_broadcast([P, GT, 4, H]),
                        in1=w2b_s[:].unsqueeze(1).to_broadcast([P, GT, 4, H]),
                        op=Alu.mult,
                    )
                    h2t = wpool.tile([P, GT, 4], dt.float32, tag="h2t")
                    nc.vector.tensor_reduce(
                        h2t[:], tmp2[:], axis=mybir.AxisListType.X, op=Alu.add
                    )
                    nc.vector.tensor_tensor(
                        out=h2all[:, g * GT : (g + 1) * GT, :],
                        in0=h2t[:],
                        in1=b2a_s[:].unsqueeze(1).to_broadcast([P, GT, 4]),
                        op=Alu.add,
                    )
                if phase == "gonly":
                    dump2(oall[:])
                    return
                nc.sync.dma_start(
                    l2slab[0:NSH, :].rearrange("(t p) f -> p t f", p=P), h2all[:]
                )
                nc.sync.dma_start(l2slab[NSH : NSH + 1, :], sent2[:])
                if phase == "gat1":
                    dump2(h2all[:, :, 0:C])
                    return
                nc.gpsimd.collective_compute(
                    "AllGather",
                    Alu.bypass,
                    replica_groups=RG,
                    ins=[l2slab[:].opt()],
                    outs=[table2[0:TROWS, :].opt()],
                )
                if phase == "ag2":
                    tt = wpool.tile([P, NT, ROW2], dt.bfloat16, tag="dumpt2")
                    nc.sync.dma_start(
                        tt[:], table2[0:NSH, :].rearrange("(t p) f -> p t f", p=P)
                    )
                    dump2(tt[:, :, 0:C])
                    return
                expand(table2, gtab2, ROW2)

                # ---- Phase D: layer-2 edge aggregation -> unnormalized out
                allout = cpool.tile([P, NT, C], dt.float32)
                for g in range(NG):
                    o2 = edge_group(gtab2, idx2_s, g, 2)
                    nc.vector.tensor_copy(allout[:, g * GT : (g + 1) * GT, :], o2[:])
                nc.sync.dma_start(
                    out_p[:].rearrange("(t p) c -> p t c", p=P), allout[:]
                )

            _emit()
    nc.compile()
    _patch_serialization(nc)
    return nc


def _prep(x, edge_index, W1, a_src1, a_dst1, b1, W2, a_src2, a_dst2, b2):
    ei = np.asarray(edge_index).astype(np.int64)
    src_all, dst_all = ei[0], ei[1]
    E = src_all.shape[0]
    counts = np.bincount(dst_all, minlength=N)
    W = 64  # pow2 ELL width (pairwise-tree friendly); widen if degree demands
    while W < int(counts.max()) + 1:
        W *= 2

    # table row position of node g: NSHE*(g//NSH) + g%NSH; sentinel at SENTROW
    g = np.arange(N, dtype=np.int64)
    pos = NSHE * (g // NSH) + (g % NSH)

    perm_e = np.argsort(dst_all, kind="stable")
    dsorted = dst_all[perm_e]
    ssorted = src_all[perm_e]
    starts = np.zeros(N + 1, np.int64)
    np.cumsum(counts, out=starts[1:])
    rank = np.arange(E, dtype=np.int64) - starts[dsorted]

    mat = np.full((N, W), SENTROW, np.int64)
    mat[:, 0] = pos
    mat[dsorted, 1 + rank] = pos[ssorted]

    # dma_gather idx order per group: i = (t_in_group*W + w)*128 + p
    idx_maps = []
    for c in range(NCORES):
        m = mat[c * NSH : (c + 1) * NSH].reshape(NT // GT, GT, P, W)
        m = m.transpose(0, 1, 3, 2).reshape(-1)  # [g, t, w, p] flattened
        idx_maps.append(np.tile(m.reshape(-1, 16).T, (8, 1)).astype(np.int16))

    bf = ml_dtypes.bfloat16
    W1aug = np.concatenate(
        [W1, (W1 @ a_src1)[:, None], (W1 @ a_dst1)[:, None]], axis=1
    ).astype(np.float32)
    w1s = (
        W1aug.reshape(KCH, P, AUG1).transpose(1, 0, 2).reshape(P, KCH * AUG1)
    ).astype(bf)
    v1 = np.concatenate([b1.astype(np.float32), np.zeros(2, np.float32)]).reshape(
        AUG1, 1
    )
    W2aug = np.concatenate(
        [W2, (W2 @ a_src2)[:, None], (W2 @ a_dst2)[:, None]], axis=1
    ).astype(np.float32)
    w2b = np.tile(W2aug.T.reshape(1, 4 * H), (P, 1)).astype(np.float32)
    b2a = np.tile(
        np.array([b2[0], b2[1], 0.0, 0.0], np.float32), (P, 1)
    ).astype(np.float32)

    xb = np.asarray(x, np.float32).astype(bf)
    in_maps = []
    for c in range(NCORES):
        in_maps.append(
            {
                "xs": np.ascontiguousarray(xb[NSH * c : NSH * (c + 1)].T),
                "w1s": w1s,
                "v1": v1,
                "idx1": idx_maps[c],
                "w2b": w2b,
                "b2a": b2a,
            }
        )
    return W, in_maps


_NC_CACHE = {}


def _get_nc(W):
    if W not in _NC_CACHE:
        _NC_CACHE[W] = _build(W)
    return _NC_CACHE[W]


def kernel(**inputs):
    from concourse.bass_utils import run_bass_kernel_spmd

    W, in_maps = _prep(
        inputs["x"], inputs["edge_index"], inputs["W1"], inputs["a_src1"],
        inputs["a_dst1"], inputs["b1"], inputs["W2"], inputs["a_src2"],
        inputs["a_dst2"], inputs["b2"],
    )
    nc = _get_nc(W)
    res = run_bass_kernel_spmd(nc, in_maps, list(range(NCORES)))
    out = np.concatenate([res.results[c]["out"] for c in range(NCORES)], axis=0)
    out = out.astype(np.float64)
    mn, mx = out.min(), out.max()
    return (2.0 * (out - mn) / (mx - mn) - 1.0).astype(np.float32)
